# revision 17
# baseline (speedup 1.0000x reference)
import sys

sys.path.insert(0, "/opt/trn_rl_repo")

import numpy as np
from contextlib import ExitStack

import ml_dtypes

import concourse.bass as bass
import concourse.mybir as mybir
import concourse.tile as tile
from concourse.bass_utils import run_bass_kernel_spmd

BF16NP = ml_dtypes.bfloat16
F32 = mybir.dt.float32
BF = mybir.dt.bfloat16
AF = mybir.ActivationFunctionType

B, S, E, D = 4, 4096, 1024, 64
SL = S // 2
NQT = 8
NKB = 16
EC = 8


def _patch_tile_drain():
    if getattr(tile.TileContext, "_drain_patched", False):
        return
    from concourse.tile import ScopedClock

    def _split_drain_and_barrier(self, tick_clock, wait_clock):
        drain_inst = self.nc.sync.drain()
        wait_clock.add_sem_waits(
            drain_inst.ins, ScopedClock({None: tick_clock.global_clock})
        )
        mi = drain_inst.ins
        si = mi.sync_info
        if si is not None and si.on_wait and len(si.on_wait) > 1:
            waits = list(si.on_wait)
            si.on_wait = waits[:1]
            for w in waits[1:]:
                d2 = self.nc.sync.drain().ins
                si2 = d2.sync_info
                if si2 is None:
                    d2.sync_info = mybir.SyncInfo(on_wait=[w], on_update=[])
                else:
                    si2.on_wait = list(si2.on_wait) + [w]
        self.nc.all_engine_barrier()
        assert self.sems is not None
        popped = self.nc._tile_sem_poison_stack.pop()
        assert popped is self._sem_poison
        self.nc.clear_and_free_semaphores(list(self.sems.allocated().values()))
        self.nc.all_engine_barrier()

    tile.TileContext._drain_and_barrier = _split_drain_and_barrier
    tile.TileContext._drain_patched = True


WAIT_LIMIT = 1


def _split_sync_waits(nc, limit=WAIT_LIMIT):
    n_nops = 0
    for f in nc.m.functions:
        for bb in f.blocks:
            il = bb.instructions
            i = 0
            while i < len(il):
                ins = il[i]
                si = ins.sync_info
                if si is not None and si.on_wait and len(si.on_wait) > limit:
                    waits = list(si.on_wait)
                    keep = waits[-limit:]
                    excess = waits[:-limit]
                    pos = i
                    for j in range(0, len(excess), limit):
                        nop = mybir.InstNoOp(
                            name=f"{ins.name}_wsplit{j}", ins=[], outs=[]
                        )
                        nop.engine = ins.engine
                        nop.sync_info = mybir.SyncInfo(
                            on_wait=excess[j : j + limit], on_update=[]
                        )
                        il.insert(pos, nop)
                        pos += 1
                        i += 1
                        n_nops += 1
                    si.on_wait = keep
                i += 1
    return n_nops


def build_nc(npairs, masked):
    _patch_tile_drain()
    nc = bass.Bass("TRN2", target_bir_lowering=False)

    qt = nc.dram_tensor("qt", [NQT, 128, EC * 512], BF, kind="ExternalInput")
    kt = nc.dram_tensor("kt", [4, 128, EC * 512], BF, kind="ExternalInput")
    vt = nc.dram_tensor("vt", [4, 128, EC * 512], BF, kind="ExternalInput")
    wq2 = nc.dram_tensor("wq2", [128, EC * 128], BF, kind="ExternalInput")
    wk2 = nc.dram_tensor("wk2", [128, EC * 128], BF, kind="ExternalInput")
    wvh = nc.dram_tensor("wvh", [128, EC * D], BF, kind="ExternalInput")
    msk = nc.dram_tensor("msk", [128, 1024], BF, kind="ExternalInput")
    o = nc.dram_tensor("o", [D + 1, S], F32, kind="ExternalOutput")

    with tile.TileContext(nc) as tc, ExitStack() as ctx:
        const = ctx.enter_context(tc.tile_pool(name="const", bufs=1))
        big = ctx.enter_context(tc.tile_pool(name="big", bufs=1))

        wq2_sb = const.tile([128, EC * 128], BF, tag="wq2")
        wk2_sb = const.tile([128, EC * 128], BF, tag="wk2")
        wvh_sb = const.tile([128, EC * D], BF, tag="wvh")
        msk_sb = const.tile([128, 1024], BF, tag="msk")
        scr = const.tile([1, 8], F32, tag="scr")

        nc.sync.dma_start(wq2_sb[:], wq2[:])
        nc.sync.dma_start(wk2_sb[:], wk2[:])
        nc.sync.dma_start(wvh_sb[:], wvh[:])
        nc.sync.dma_start(msk_sb[:], msk[:])

        nc.vector.memset(scr[:], 0.0)
        nc.scalar.activation(scr[:], scr[:], AF.Exp)

        kT_t = [
            big.tile([128, EC * 512], BF, tag=f"kT{t}", name=f"kT{t}")
            for t in range(4)
        ]
        vT_q = [
            big.tile([128, EC * 512], BF, tag=f"vT{t}", name=f"vT{t}")
            for t in range(4)
        ]
        qT_g = [
            big.tile([128, EC * 512], BF, tag=f"qT{g}", name=f"qT{g}")
            for g in range(NQT)
        ]

        qhT2 = big.tile([128, S], BF, tag="qhT2")
        khT2 = big.tile([128, SL // 2], BF, tag="khT2")
        vh_sb = big.tile([128, NKB * (D + 1)], BF, tag="vh")
        ob = big.tile([128, S], F32, tag="ob")

        nc.vector.memset(
            vh_sb[:].rearrange("p (b c) -> p b c", c=D + 1)[:, :, D], 1.0
        )

        if masked:
            nc.sync.dma_start(qT_g[0][:], qt[0])
            nc.sync.dma_start(kT_t[0][:], kt[0])
            nc.sync.dma_start(vT_q[0][:], vt[0])
            nc.sync.dma_start(qT_g[1][:], qt[1])
            nc.sync.dma_start(vT_q[1][:], vt[1])
            nc.sync.dma_start(kT_t[1][:], kt[1])
            nc.sync.dma_start(qT_g[2][:], qt[2])
            nc.sync.dma_start(qT_g[3][:], qt[3])
            nc.sync.dma_start(vT_q[2][:], vt[2])
            nc.sync.dma_start(kT_t[2][:], kt[2])
            nc.sync.dma_start(qT_g[4][:], qt[4])
            nc.sync.dma_start(qT_g[5][:], qt[5])
            nc.sync.dma_start(vT_q[3][:], vt[3])
            nc.sync.dma_start(qT_g[6][:], qt[6])
            nc.sync.dma_start(kT_t[3][:], kt[3])
            nc.sync.dma_start(qT_g[7][:], qt[7])
        else:
            for t in range(4):
                nc.sync.dma_start(kT_t[t][:], kt[t])
            for t in range(4):
                nc.sync.dma_start(vT_q[t][:], vt[t])
            for g in range(NQT):
                nc.sync.dma_start(qT_g[g][:], qt[g])

        ptpool = ctx.enter_context(tc.tile_pool(name="pt", bufs=4))
        psS = ctx.enter_context(tc.tile_pool(name="psS", bufs=2, space="PSUM"))
        psP = ctx.enter_context(tc.tile_pool(name="psP", bufs=2, space="PSUM"))
        psO = ctx.enter_context(tc.tile_pool(name="psO", bufs=2, space="PSUM"))

        def khT_proj(t):
            ps = psS.tile([128, 512], F32, tag="psS")
            for c in range(EC):
                nc.tensor.matmul(
                    ps[:],
                    lhsT=wk2_sb[:, c * 128 : (c + 1) * 128],
                    rhs=kT_t[t][:].rearrange("p (c f) -> p c f", c=EC)[:, c, :],
                    start=(c == 0),
                    stop=(c == EC - 1),
                )
            pv = ps[:].rearrange("p (u f) -> p u f", f=128)
            nc.vector.tensor_copy(
                khT2[0:64, t * 256 : (t + 1) * 256]
                .rearrange("p (u f) -> p u f", f=128),
                pv[0:64, 0:4:2, :],
            )
            nc.vector.tensor_copy(
                khT2[64:128, t * 256 : (t + 1) * 256]
                .rearrange("p (u f) -> p u f", f=128),
                pv[64:128, 1:4:2, :],
            )

        def qproj(g):
            ps = psS.tile([128, 512], F32, tag="psS")
            for c in range(EC):
                nc.tensor.matmul(
                    ps[:],
                    lhsT=wq2_sb[:, c * 128 : (c + 1) * 128],
                    rhs=qT_g[g][:].rearrange("p (c f) -> p c f", c=EC)[:, c, :],
                    start=(c == 0),
                    stop=(c == EC - 1),
                )
            nc.vector.tensor_copy(qhT2[:, g * 512 : (g + 1) * 512], ps[:])

        def vh_pair(j):
            for bi in range(2):
                b = 2 * j + bi
                ps = psS.tile([128, 512], F32, tag="psS")
                for c in range(EC):
                    nc.tensor.matmul(
                        ps[:, 0:D],
                        lhsT=vT_q[b // 4][:].rearrange("p (c f) -> p c f", c=EC)[
                            :, c, (b % 4) * 128 : (b % 4 + 1) * 128
                        ],
                        rhs=wvh_sb[:, c * D : (c + 1) * D],
                        start=(c == 0),
                        stop=(c == EC - 1),
                    )
                nc.vector.tensor_copy(
                    vh_sb[:, b * (D + 1) : b * (D + 1) + D], ps[:, 0:D]
                )

        wm = psS.tile([128, 512], F32, tag="psS", name="wm")
        for w in range(26):
            nc.tensor.matmul(
                wm[:, 0:128],
                lhsT=wq2_sb[:, 0:128],
                rhs=wq2_sb[:, 0:128],
                start=True,
                stop=True,
            )

        qproj(0)
        khT_done = set()
        vh_done = set()

        flat = [(g, j) for g in range(NQT) for j in range(npairs[g])]
        pts = {}
        po_t = {}
        sc_cur = 0

        def scores(p):
            g, j = flat[p]
            t = j // 2
            if t not in khT_done:
                khT_done.add(t)
                khT_proj(t)
            pp = psP.tile([128, 1024], F32, tag="psP")
            nc.tensor.matmul(
                pp[:, 0:512],
                lhsT=khT2[0:64, j * 128 : (j + 1) * 128],
                rhs=qhT2[0:64, g * 512 : (g + 1) * 512],
                start=True,
                stop=True,
            )
            nc.tensor.matmul(
                pp[:, 512:1024],
                lhsT=khT2[64:128, j * 128 : (j + 1) * 128],
                rhs=qhT2[64:128, g * 512 : (g + 1) * 512],
                start=True,
                stop=True,
            )
            pt = ptpool.tile([128, 1024], BF, tag="pt")
            nc.scalar.activation(pt[:], pp[:], AF.Exp)
            if masked and j == npairs[g] - 1:
                nc.vector.tensor_mul(pt[:], pt[:], msk_sb[:])
            pts[p] = pt

        def av(p):
            g, j = flat[p]
            if j not in vh_done:
                vh_done.add(j)
                vh_pair(j)
            pt = pts.pop(p)
            po = po_t[g]
            nc.tensor.matmul(
                po[0:65, :],
                lhsT=vh_sb[:, (2 * j) * 65 : (2 * j + 1) * 65],
                rhs=pt[:, 0:512],
                start=(j == 0),
                stop=False,
            )
            nc.tensor.matmul(
                po[0:65, :],
                lhsT=vh_sb[:, (2 * j + 1) * 65 : (2 * j + 2) * 65],
                rhs=pt[:, 512:1024],
                start=False,
                stop=(j == npairs[g] - 1),
            )

        khT_done.add(0)
        khT_proj(0)
        vh_done.add(0)
        vh_pair(0)

        def keep_warm(n):
            kw = psS.tile([128, 512], F32, tag="psS", name="kw")
            for w in range(n):
                nc.tensor.matmul(
                    kw[:, 0:64],
                    lhsT=wq2_sb[:, 0:128],
                    rhs=wq2_sb[:, 0:64],
                    start=True,
                    stop=True,
                )

        def tile_start(g):
            po_t[g] = psO.tile([128, 512], F32, tag="psO", name=f"po{g}")
            if g in (1, 2, 3, 4):
                keep_warm(4)
            if g + 1 < NQT:
                qproj(g + 1)

        def tile_end(g):
            nc.vector.tensor_copy(
                ob[0:65, g * 512 : (g + 1) * 512], po_t[g][0:65, :]
            )
            nc.sync.dma_start(
                o[:, g * 512 : (g + 1) * 512],
                ob[0:65, g * 512 : (g + 1) * 512],
            )

        tile_start(0)
        scores(0)
        if len(flat) > 1:
            if flat[1][1] == 0:
                tile_start(flat[1][0])
            scores(1)
        sc_cur = 2

        for p in range(0, len(flat), 2):
            grp = [x for x in (p, p + 1) if x < len(flat)]
            for x in (p + 2, p + 3):
                if x < len(flat):
                    if flat[x][1] == 0:
                        tile_start(flat[x][0])
                    scores(x)
                    sc_cur = x + 1
            for x in grp:
                g, j = flat[x]
                av(x)
                if j == npairs[g] - 1:
                    tile_end(g)

    _split_sync_waits(nc)
    return nc


_CACHE = {}


def _get_nc(causal):
    key = bool(causal)
    if key not in _CACHE:
        if causal:
            npairs = [g + 1 for g in range(NQT)]
        else:
            npairs = [NKB // 2] * NQT
        _CACHE[key] = build_nc(npairs, key)
    return _CACHE[key]


def kernel(q, k, v, mask, wq, wk, wv):
    q = np.asarray(q, np.float32)
    k = np.asarray(k, np.float32)
    v = np.asarray(v, np.float32)
    mask = np.asarray(mask)
    wq = np.asarray(wq, np.float32)
    wk = np.asarray(wk, np.float32)
    wv = np.asarray(wv, np.float32)

    m0 = mask[0]
    causal = bool(m0[0, 1] == 0)
    tril = np.tril(np.ones((S, S), np.int32))
    if causal:
        ok = np.array_equal(m0.astype(np.int32), tril)
    else:
        ok = bool((m0 != 0).all())
    if not ok:
        qh = q @ wq
        kh = k @ wk
        vh = v @ wv
        s = np.einsum("bqd,bkd->bqk", qh, kh) / np.sqrt(D)
        s = np.where(mask == 0, -np.inf, s)
        s = s - s.max(-1, keepdims=True)
        p = np.exp(s)
        p /= p.sum(-1, keepdims=True)
        return np.einsum("bqk,bkd->bqd", p, vh).astype(np.float32)

    nc = _get_nc(causal)

    def dup_w(w, scale=1.0):
        wc = (w * scale).reshape(EC, 128, D).astype(BF16NP)
        out = np.empty((128, EC, 128), dtype=BF16NP)
        out[:, :, 0:D] = wc.transpose(1, 0, 2)
        out[:, :, D:128] = wc.transpose(1, 0, 2)
        return np.ascontiguousarray(out.reshape(128, EC * 128))

    wq2 = dup_w(wq, 1.0 / np.sqrt(D))
    wk2 = dup_w(wk)
    wvh = np.ascontiguousarray(
        wv.reshape(EC, 128, D).transpose(1, 0, 2).reshape(128, EC * D)
    ).astype(BF16NP)

    def pack_tiles(x):
        nt = x.shape[0] // 512
        t4 = x.reshape(nt, 512, EC, 128)
        return np.ascontiguousarray(t4.transpose(0, 3, 2, 1)).reshape(
            nt, 128, EC * 512
        )

    in_maps = []
    for b in range(B):
        qT = pack_tiles(q[b].astype(BF16NP))
        for p in range(2):
            kb = k[b].reshape(32, 128, E)[p::2].reshape(SL, E)
            vb = v[b].reshape(32, 128, E)[p::2].reshape(SL, E)
            kT = pack_tiles(kb.astype(BF16NP))
            vT = pack_tiles(vb.astype(BF16NP))
            if causal:
                kk = np.arange(128)[:, None]
                qq = np.arange(512)[None, :]
                m0b = (qq >= kk + 128 * p).astype(BF16NP)
                m1b = (qq >= kk + 128 * p + 256).astype(BF16NP)
                mskd = np.concatenate([m0b, m1b], axis=1)
            else:
                mskd = np.ones((128, 1024), BF16NP)
            in_maps.append(
                {
                    "qt": qT,
                    "kt": kT,
                    "vt": vT,
                    "wq2": wq2,
                    "wk2": wk2,
                    "wvh": wvh,
                    "msk": mskd,
                }
            )

    globals()["_last_in_maps"] = in_maps
    res = run_bass_kernel_spmd(nc, in_maps, core_ids=list(range(8)))

    out = np.empty((B, S, D), np.float32)
    for b in range(B):
        oe = res.results[2 * b]["o"]
        oo = res.results[2 * b + 1]["o"]
        num = oe[0:D] + oo[0:D]
        den = oe[D : D + 1] + oo[D : D + 1]
        out[b] = (num / den).T
    return out


# revision 23
# speedup vs baseline: 1.0210x; 1.0210x over previous
import sys

sys.path.insert(0, "/opt/trn_rl_repo")

import numpy as np
from contextlib import ExitStack

import ml_dtypes

import concourse.bass as bass
import concourse.mybir as mybir
import concourse.tile as tile
from concourse.bass_utils import run_bass_kernel_spmd

BF16NP = ml_dtypes.bfloat16
F32 = mybir.dt.float32
BF = mybir.dt.bfloat16
AF = mybir.ActivationFunctionType

B, S, E, D = 4, 4096, 1024, 64
SL = S // 2
NQT = 8
NKB = 16
EC = 8


def _patch_tile_drain():
    if getattr(tile.TileContext, "_drain_patched", False):
        return
    from concourse.tile import ScopedClock

    def _split_drain_and_barrier(self, tick_clock, wait_clock):
        drain_inst = self.nc.sync.drain()
        wait_clock.add_sem_waits(
            drain_inst.ins, ScopedClock({None: tick_clock.global_clock})
        )
        mi = drain_inst.ins
        si = mi.sync_info
        if si is not None and si.on_wait and len(si.on_wait) > 1:
            waits = list(si.on_wait)
            si.on_wait = waits[:1]
            for w in waits[1:]:
                d2 = self.nc.sync.drain().ins
                si2 = d2.sync_info
                if si2 is None:
                    d2.sync_info = mybir.SyncInfo(on_wait=[w], on_update=[])
                else:
                    si2.on_wait = list(si2.on_wait) + [w]
        self.nc.all_engine_barrier()
        assert self.sems is not None
        popped = self.nc._tile_sem_poison_stack.pop()
        assert popped is self._sem_poison
        self.nc.clear_and_free_semaphores(list(self.sems.allocated().values()))
        self.nc.all_engine_barrier()

    tile.TileContext._drain_and_barrier = _split_drain_and_barrier
    tile.TileContext._drain_patched = True


WAIT_LIMIT = 1


def _split_sync_waits(nc, limit=WAIT_LIMIT):
    n_nops = 0
    for f in nc.m.functions:
        for bb in f.blocks:
            il = bb.instructions
            i = 0
            while i < len(il):
                ins = il[i]
                si = ins.sync_info
                if si is not None and si.on_wait and len(si.on_wait) > limit:
                    waits = list(si.on_wait)
                    keep = waits[-limit:]
                    excess = waits[:-limit]
                    pos = i
                    for j in range(0, len(excess), limit):
                        nop = mybir.InstNoOp(
                            name=f"{ins.name}_wsplit{j}", ins=[], outs=[]
                        )
                        nop.engine = ins.engine
                        nop.sync_info = mybir.SyncInfo(
                            on_wait=excess[j : j + limit], on_update=[]
                        )
                        il.insert(pos, nop)
                        pos += 1
                        i += 1
                        n_nops += 1
                    si.on_wait = keep
                i += 1
    return n_nops


def build_nc(npairs, masked):
    _patch_tile_drain()
    nc = bass.Bass("TRN2", target_bir_lowering=False)

    qt = nc.dram_tensor("qt", [NQT, 128, EC * 512], BF, kind="ExternalInput")
    kt = nc.dram_tensor("kt", [4, 128, EC * 512], BF, kind="ExternalInput")
    vt = nc.dram_tensor("vt", [4, 128, EC * 512], BF, kind="ExternalInput")
    wq2 = nc.dram_tensor("wq2", [128, EC * 128], BF, kind="ExternalInput")
    wk2 = nc.dram_tensor("wk2", [128, EC * 128], BF, kind="ExternalInput")
    wvh = nc.dram_tensor("wvh", [128, EC * D], BF, kind="ExternalInput")
    msk = nc.dram_tensor("msk", [128, 1024], BF, kind="ExternalInput")
    o = nc.dram_tensor("o", [D + 1, S], F32, kind="ExternalOutput")

    with tile.TileContext(nc) as tc, ExitStack() as ctx:
        const = ctx.enter_context(tc.tile_pool(name="const", bufs=1))
        big = ctx.enter_context(tc.tile_pool(name="big", bufs=1))

        wq2_sb = const.tile([128, EC * 128], BF, tag="wq2")
        wk2_sb = const.tile([128, EC * 128], BF, tag="wk2")
        wvh_sb = const.tile([128, EC * D], BF, tag="wvh")
        msk_sb = const.tile([128, 1024], BF, tag="msk")
        scr = const.tile([1, 8], F32, tag="scr")

        nc.sync.dma_start(wq2_sb[:], wq2[:])

        nc.vector.memset(scr[:], 0.0)
        nc.scalar.activation(scr[:], scr[:], AF.Exp)

        kT_t = [
            big.tile([128, EC * 512], BF, tag=f"kT{t}", name=f"kT{t}")
            for t in range(4)
        ]
        vT_q = [
            big.tile([128, EC * 512], BF, tag=f"vT{t}", name=f"vT{t}")
            for t in range(4)
        ]
        qT_g = [
            big.tile([128, EC * 512], BF, tag=f"qT{g}", name=f"qT{g}")
            for g in range(NQT)
        ]

        qhT2 = big.tile([128, S], BF, tag="qhT2")
        khT2 = big.tile([128, SL // 2], BF, tag="khT2")
        vh_sb = big.tile([128, NKB * (D + 1)], BF, tag="vh")
        ob = big.tile([128, S], F32, tag="ob")

        nc.vector.memset(
            vh_sb[:].rearrange("p (b c) -> p b c", c=D + 1)[:, :, D], 1.0
        )

        if masked:
            nc.sync.dma_start(qT_g[0][:], qt[0])
            nc.sync.dma_start(kT_t[0][:], kt[0])
            nc.sync.dma_start(wk2_sb[:], wk2[:])
            nc.sync.dma_start(wvh_sb[:], wvh[:])
            nc.sync.dma_start(msk_sb[:], msk[:])
            nc.sync.dma_start(vT_q[0][:], vt[0])
            nc.sync.dma_start(qT_g[1][:], qt[1])
            nc.sync.dma_start(vT_q[1][:], vt[1])
            nc.sync.dma_start(kT_t[1][:], kt[1])
            nc.sync.dma_start(qT_g[2][:], qt[2])
            nc.sync.dma_start(qT_g[3][:], qt[3])
            nc.sync.dma_start(vT_q[2][:], vt[2])
            nc.sync.dma_start(kT_t[2][:], kt[2])
            nc.sync.dma_start(qT_g[4][:], qt[4])
            nc.sync.dma_start(qT_g[5][:], qt[5])
            nc.sync.dma_start(vT_q[3][:], vt[3])
            nc.sync.dma_start(qT_g[6][:], qt[6])
            nc.sync.dma_start(kT_t[3][:], kt[3])
            nc.sync.dma_start(qT_g[7][:], qt[7])
        else:
            nc.sync.dma_start(wk2_sb[:], wk2[:])
            nc.sync.dma_start(wvh_sb[:], wvh[:])
            nc.sync.dma_start(msk_sb[:], msk[:])
            for t in range(4):
                nc.sync.dma_start(kT_t[t][:], kt[t])
            for t in range(4):
                nc.sync.dma_start(vT_q[t][:], vt[t])
            for g in range(NQT):
                nc.sync.dma_start(qT_g[g][:], qt[g])

        ptpool = ctx.enter_context(tc.tile_pool(name="pt", bufs=4))
        psS = ctx.enter_context(tc.tile_pool(name="psS", bufs=2, space="PSUM"))
        psP = ctx.enter_context(tc.tile_pool(name="psP", bufs=2, space="PSUM"))
        psO = ctx.enter_context(tc.tile_pool(name="psO", bufs=2, space="PSUM"))

        def khT_proj(t):
            ps = psS.tile([128, 512], F32, tag="psS")
            for c in range(EC):
                nc.tensor.matmul(
                    ps[:],
                    lhsT=wk2_sb[:, c * 128 : (c + 1) * 128],
                    rhs=kT_t[t][:].rearrange("p (c f) -> p c f", c=EC)[:, c, :],
                    start=(c == 0),
                    stop=(c == EC - 1),
                )
            pv = ps[:].rearrange("p (u f) -> p u f", f=128)
            nc.vector.tensor_copy(
                khT2[0:64, t * 256 : (t + 1) * 256]
                .rearrange("p (u f) -> p u f", f=128),
                pv[0:64, 0:4:2, :],
            )
            nc.vector.tensor_copy(
                khT2[64:128, t * 256 : (t + 1) * 256]
                .rearrange("p (u f) -> p u f", f=128),
                pv[64:128, 1:4:2, :],
            )

        def qproj(g):
            ps = psS.tile([128, 512], F32, tag="psS")
            for c in range(EC):
                nc.tensor.matmul(
                    ps[:],
                    lhsT=wq2_sb[:, c * 128 : (c + 1) * 128],
                    rhs=qT_g[g][:].rearrange("p (c f) -> p c f", c=EC)[:, c, :],
                    start=(c == 0),
                    stop=(c == EC - 1),
                )
            nc.vector.tensor_copy(qhT2[:, g * 512 : (g + 1) * 512], ps[:])

        def vh_pair(j):
            for bi in range(2):
                b = 2 * j + bi
                ps = psS.tile([128, 512], F32, tag="psS")
                for c in range(EC):
                    nc.tensor.matmul(
                        ps[:, 0:D],
                        lhsT=vT_q[b // 4][:].rearrange("p (c f) -> p c f", c=EC)[
                            :, c, (b % 4) * 128 : (b % 4 + 1) * 128
                        ],
                        rhs=wvh_sb[:, c * D : (c + 1) * D],
                        start=(c == 0),
                        stop=(c == EC - 1),
                    )
                nc.vector.tensor_copy(
                    vh_sb[:, b * (D + 1) : b * (D + 1) + D], ps[:, 0:D]
                )

        wm = psS.tile([128, 512], F32, tag="psS", name="wm")
        for w in range(26):
            nc.tensor.matmul(
                wm[:, 0:128],
                lhsT=wq2_sb[:, 0:128],
                rhs=wq2_sb[:, 0:128],
                start=True,
                stop=True,
            )

        qproj_done = {0}
        qproj(0)
        khT_done = set()
        vh_done = set()

        flat = [(g, j) for g in range(NQT) for j in range(npairs[g])]
        pts = {}
        po_t = {}
        sc_cur = 0

        def scores(p):
            g, j = flat[p]
            if g not in qproj_done:
                qproj_done.add(g)
                qproj(g)
            t = j // 2
            if t not in khT_done:
                khT_done.add(t)
                khT_proj(t)
            pp = psP.tile([128, 1024], F32, tag="psP")
            nc.tensor.matmul(
                pp[:, 0:512],
                lhsT=khT2[0:64, j * 128 : (j + 1) * 128],
                rhs=qhT2[0:64, g * 512 : (g + 1) * 512],
                start=True,
                stop=True,
            )
            nc.tensor.matmul(
                pp[:, 512:1024],
                lhsT=khT2[64:128, j * 128 : (j + 1) * 128],
                rhs=qhT2[64:128, g * 512 : (g + 1) * 512],
                start=True,
                stop=True,
            )
            pt = ptpool.tile([128, 1024], BF, tag="pt")
            nc.scalar.activation(pt[:], pp[:], AF.Exp)
            if masked and j == npairs[g] - 1:
                nc.vector.tensor_mul(pt[:], pt[:], msk_sb[:])
            pts[p] = pt

        def av(p):
            g, j = flat[p]
            if j not in vh_done:
                vh_done.add(j)
                vh_pair(j)
            pt = pts.pop(p)
            po = po_t[g]
            nc.tensor.matmul(
                po[0:65, :],
                lhsT=vh_sb[:, (2 * j) * 65 : (2 * j + 1) * 65],
                rhs=pt[:, 0:512],
                start=(j == 0),
                stop=False,
            )
            nc.tensor.matmul(
                po[0:65, :],
                lhsT=vh_sb[:, (2 * j + 1) * 65 : (2 * j + 2) * 65],
                rhs=pt[:, 512:1024],
                start=False,
                stop=(j == npairs[g] - 1),
            )

        khT_done.add(0)
        khT_proj(0)
        vh_done.add(0)
        vh_pair(0)

        def tile_start(g):
            po_t[g] = psO.tile([128, 512], F32, tag="psO", name=f"po{g}")

        def tile_end(g):
            nc.vector.tensor_copy(
                ob[0:65, g * 512 : (g + 1) * 512], po_t[g][0:65, :]
            )
            nc.sync.dma_start(
                o[:, g * 512 : (g + 1) * 512],
                ob[0:65, g * 512 : (g + 1) * 512],
            )

        tile_start(0)
        scores(0)
        if len(flat) > 1:
            if flat[1][1] == 0:
                tile_start(flat[1][0])
            scores(1)
        sc_cur = 2

        for p in range(0, len(flat), 2):
            grp = [x for x in (p, p + 1) if x < len(flat)]
            for x in (p + 2, p + 3):
                if x < len(flat):
                    if flat[x][1] == 0:
                        tile_start(flat[x][0])
                    scores(x)
                    sc_cur = x + 1
            for x in grp:
                g, j = flat[x]
                av(x)
                if j == npairs[g] - 1:
                    tile_end(g)

    _split_sync_waits(nc)
    return nc


_CACHE = {}


def _get_nc(causal):
    key = bool(causal)
    if key not in _CACHE:
        if causal:
            npairs = [g + 1 for g in range(NQT)]
        else:
            npairs = [NKB // 2] * NQT
        _CACHE[key] = build_nc(npairs, key)
    return _CACHE[key]


def kernel(q, k, v, mask, wq, wk, wv):
    q = np.asarray(q, np.float32)
    k = np.asarray(k, np.float32)
    v = np.asarray(v, np.float32)
    mask = np.asarray(mask)
    wq = np.asarray(wq, np.float32)
    wk = np.asarray(wk, np.float32)
    wv = np.asarray(wv, np.float32)

    m0 = mask[0]
    causal = bool(m0[0, 1] == 0)
    tril = np.tril(np.ones((S, S), np.int32))
    if causal:
        ok = np.array_equal(m0.astype(np.int32), tril)
    else:
        ok = bool((m0 != 0).all())
    if not ok:
        qh = q @ wq
        kh = k @ wk
        vh = v @ wv
        s = np.einsum("bqd,bkd->bqk", qh, kh) / np.sqrt(D)
        s = np.where(mask == 0, -np.inf, s)
        s = s - s.max(-1, keepdims=True)
        p = np.exp(s)
        p /= p.sum(-1, keepdims=True)
        return np.einsum("bqk,bkd->bqd", p, vh).astype(np.float32)

    nc = _get_nc(causal)

    def dup_w(w, scale=1.0):
        wc = (w * scale).reshape(EC, 128, D).astype(BF16NP)
        out = np.empty((128, EC, 128), dtype=BF16NP)
        out[:, :, 0:D] = wc.transpose(1, 0, 2)
        out[:, :, D:128] = wc.transpose(1, 0, 2)
        return np.ascontiguousarray(out.reshape(128, EC * 128))

    wq2 = dup_w(wq, 1.0 / np.sqrt(D))
    wk2 = dup_w(wk)
    wvh = np.ascontiguousarray(
        wv.reshape(EC, 128, D).transpose(1, 0, 2).reshape(128, EC * D)
    ).astype(BF16NP)

    def pack_tiles(x):
        nt = x.shape[0] // 512
        t4 = x.reshape(nt, 512, EC, 128)
        return np.ascontiguousarray(t4.transpose(0, 3, 2, 1)).reshape(
            nt, 128, EC * 512
        )

    in_maps = []
    for b in range(B):
        qT = pack_tiles(q[b].astype(BF16NP))
        for p in range(2):
            kb = k[b].reshape(32, 128, E)[p::2].reshape(SL, E)
            vb = v[b].reshape(32, 128, E)[p::2].reshape(SL, E)
            kT = pack_tiles(kb.astype(BF16NP))
            vT = pack_tiles(vb.astype(BF16NP))
            if causal:
                kk = np.arange(128)[:, None]
                qq = np.arange(512)[None, :]
                m0b = (qq >= kk + 128 * p).astype(BF16NP)
                m1b = (qq >= kk + 128 * p + 256).astype(BF16NP)
                mskd = np.concatenate([m0b, m1b], axis=1)
            else:
                mskd = np.ones((128, 1024), BF16NP)
            in_maps.append(
                {
                    "qt": qT,
                    "kt": kT,
                    "vt": vT,
                    "wq2": wq2,
                    "wk2": wk2,
                    "wvh": wvh,
                    "msk": mskd,
                }
            )

    globals()["_last_in_maps"] = in_maps
    res = run_bass_kernel_spmd(nc, in_maps, core_ids=list(range(8)))

    out = np.empty((B, S, D), np.float32)
    for b in range(B):
        oe = res.results[2 * b]["o"]
        oo = res.results[2 * b + 1]["o"]
        num = oe[0:D] + oo[0:D]
        den = oe[D : D + 1] + oo[D : D + 1]
        out[b] = (num / den).T
    return out


# revision 25
# speedup vs baseline: 1.0647x; 1.0427x over previous
import sys

sys.path.insert(0, "/opt/trn_rl_repo")

import numpy as np
from contextlib import ExitStack

import ml_dtypes

import concourse.bass as bass
import concourse.mybir as mybir
import concourse.tile as tile
from concourse.bass_utils import run_bass_kernel_spmd

BF16NP = ml_dtypes.bfloat16
F32 = mybir.dt.float32
BF = mybir.dt.bfloat16
AF = mybir.ActivationFunctionType

B, S, E, D = 4, 4096, 1024, 64
SL = S // 2
NQT = 8
NKB = 16
EC = 8


def _patch_tile_drain():
    if getattr(tile.TileContext, "_drain_patched", False):
        return
    from concourse.tile import ScopedClock

    def _split_drain_and_barrier(self, tick_clock, wait_clock):
        drain_inst = self.nc.sync.drain()
        wait_clock.add_sem_waits(
            drain_inst.ins, ScopedClock({None: tick_clock.global_clock})
        )
        mi = drain_inst.ins
        si = mi.sync_info
        if si is not None and si.on_wait and len(si.on_wait) > 1:
            waits = list(si.on_wait)
            si.on_wait = waits[:1]
            for w in waits[1:]:
                d2 = self.nc.sync.drain().ins
                si2 = d2.sync_info
                if si2 is None:
                    d2.sync_info = mybir.SyncInfo(on_wait=[w], on_update=[])
                else:
                    si2.on_wait = list(si2.on_wait) + [w]
        self.nc.all_engine_barrier()
        assert self.sems is not None
        popped = self.nc._tile_sem_poison_stack.pop()
        assert popped is self._sem_poison
        self.nc.clear_and_free_semaphores(list(self.sems.allocated().values()))
        self.nc.all_engine_barrier()

    tile.TileContext._drain_and_barrier = _split_drain_and_barrier
    tile.TileContext._drain_patched = True


WAIT_LIMIT = 1


def _split_sync_waits(nc, limit=WAIT_LIMIT):
    n_nops = 0
    for f in nc.m.functions:
        for bb in f.blocks:
            il = bb.instructions
            i = 0
            while i < len(il):
                ins = il[i]
                si = ins.sync_info
                if si is not None and si.on_wait and len(si.on_wait) > limit:
                    waits = list(si.on_wait)
                    keep = waits[-limit:]
                    excess = waits[:-limit]
                    pos = i
                    for j in range(0, len(excess), limit):
                        nop = mybir.InstNoOp(
                            name=f"{ins.name}_wsplit{j}", ins=[], outs=[]
                        )
                        nop.engine = ins.engine
                        nop.sync_info = mybir.SyncInfo(
                            on_wait=excess[j : j + limit], on_update=[]
                        )
                        il.insert(pos, nop)
                        pos += 1
                        i += 1
                        n_nops += 1
                    si.on_wait = keep
                i += 1
    return n_nops


def build_nc(npairs, masked):
    _patch_tile_drain()
    nc = bass.Bass("TRN2", target_bir_lowering=False)

    qt = nc.dram_tensor("qt", [NQT, 128, EC * 512], BF, kind="ExternalInput")
    kt = nc.dram_tensor("kt", [4, 128, EC * 512], BF, kind="ExternalInput")
    vt = nc.dram_tensor("vt", [4, 128, EC * 512], BF, kind="ExternalInput")
    wq2 = nc.dram_tensor("wq2", [128, EC * 128], BF, kind="ExternalInput")
    wk2 = nc.dram_tensor("wk2", [128, EC * 128], BF, kind="ExternalInput")
    wvh = nc.dram_tensor("wvh", [128, EC * D], BF, kind="ExternalInput")
    msk = nc.dram_tensor("msk", [128, 1024], BF, kind="ExternalInput")
    o = nc.dram_tensor("o", [D + 1, S], F32, kind="ExternalOutput")

    with tile.TileContext(nc) as tc, ExitStack() as ctx:
        const = ctx.enter_context(tc.tile_pool(name="const", bufs=1))
        big = ctx.enter_context(tc.tile_pool(name="big", bufs=1))

        wq2_sb = const.tile([128, EC * 128], BF, tag="wq2")
        wk2_sb = const.tile([128, EC * 128], BF, tag="wk2")
        wvh_sb = const.tile([128, EC * D], BF, tag="wvh")
        msk_sb = const.tile([128, 1024], BF, tag="msk")
        scr = const.tile([1, 8], F32, tag="scr")

        nc.sync.dma_start(wq2_sb[:], wq2[:])

        nc.vector.memset(scr[:], 0.0)
        nc.scalar.activation(scr[:], scr[:], AF.Exp)

        kT_t = [
            big.tile([128, EC * 512], BF, tag=f"kT{t}", name=f"kT{t}")
            for t in range(4)
        ]
        vT_q = [
            big.tile([128, EC * 512], BF, tag=f"vT{t}", name=f"vT{t}")
            for t in range(4)
        ]
        qT_g = [
            big.tile([128, EC * 512], BF, tag=f"qT{g}", name=f"qT{g}")
            for g in range(NQT)
        ]

        qhT2 = big.tile([128, S], BF, tag="qhT2")
        khT2 = big.tile([128, SL // 2], BF, tag="khT2")
        vh_sb = big.tile([128, NKB * (D + 1)], BF, tag="vh")
        ob = big.tile([128, S], F32, tag="ob")

        nc.vector.memset(
            vh_sb[:].rearrange("p (b c) -> p b c", c=D + 1)[:, :, D], 1.0
        )

        H = EC * 512 // 2

        def dma2(sb, dram_slab):
            nc.sync.dma_start(sb[:, 0:H], dram_slab[:, 0:H])
            nc.sync.dma_start(sb[:, H : 2 * H], dram_slab[:, H : 2 * H])

        if masked:
            dma2(qT_g[0], qt[0])
            dma2(kT_t[0], kt[0])
            nc.sync.dma_start(wk2_sb[:], wk2[:])
            nc.sync.dma_start(wvh_sb[:], wvh[:])
            nc.sync.dma_start(msk_sb[:], msk[:])
            dma2(vT_q[0], vt[0])
            dma2(qT_g[1], qt[1])
            dma2(vT_q[1], vt[1])
            dma2(kT_t[1], kt[1])
            dma2(qT_g[2], qt[2])
            dma2(qT_g[3], qt[3])
            dma2(vT_q[2], vt[2])
            dma2(kT_t[2], kt[2])
            dma2(qT_g[4], qt[4])
            dma2(qT_g[5], qt[5])
            dma2(vT_q[3], vt[3])
            dma2(qT_g[6], qt[6])
            dma2(kT_t[3], kt[3])
            dma2(qT_g[7], qt[7])
        else:
            nc.sync.dma_start(wk2_sb[:], wk2[:])
            nc.sync.dma_start(wvh_sb[:], wvh[:])
            nc.sync.dma_start(msk_sb[:], msk[:])
            for t in range(4):
                nc.sync.dma_start(kT_t[t][:], kt[t])
            for t in range(4):
                nc.sync.dma_start(vT_q[t][:], vt[t])
            for g in range(NQT):
                nc.sync.dma_start(qT_g[g][:], qt[g])

        ptpool = ctx.enter_context(tc.tile_pool(name="pt", bufs=4))
        psS = ctx.enter_context(tc.tile_pool(name="psS", bufs=2, space="PSUM"))
        psP = ctx.enter_context(tc.tile_pool(name="psP", bufs=2, space="PSUM"))
        psO = ctx.enter_context(tc.tile_pool(name="psO", bufs=2, space="PSUM"))

        def khT_proj(t):
            ps = psS.tile([128, 512], F32, tag="psS")
            for c in range(EC):
                nc.tensor.matmul(
                    ps[:],
                    lhsT=wk2_sb[:, c * 128 : (c + 1) * 128],
                    rhs=kT_t[t][:].rearrange("p (c f) -> p c f", c=EC)[:, c, :],
                    start=(c == 0),
                    stop=(c == EC - 1),
                )
            pv = ps[:].rearrange("p (u f) -> p u f", f=128)
            nc.vector.tensor_copy(
                khT2[0:64, t * 256 : (t + 1) * 256]
                .rearrange("p (u f) -> p u f", f=128),
                pv[0:64, 0:4:2, :],
            )
            nc.vector.tensor_copy(
                khT2[64:128, t * 256 : (t + 1) * 256]
                .rearrange("p (u f) -> p u f", f=128),
                pv[64:128, 1:4:2, :],
            )

        def qproj(g):
            ps = psS.tile([128, 512], F32, tag="psS")
            for c in range(EC):
                nc.tensor.matmul(
                    ps[:],
                    lhsT=wq2_sb[:, c * 128 : (c + 1) * 128],
                    rhs=qT_g[g][:].rearrange("p (c f) -> p c f", c=EC)[:, c, :],
                    start=(c == 0),
                    stop=(c == EC - 1),
                )
            nc.vector.tensor_copy(qhT2[:, g * 512 : (g + 1) * 512], ps[:])

        def vh_pair(j):
            for bi in range(2):
                b = 2 * j + bi
                ps = psS.tile([128, 512], F32, tag="psS")
                for c in range(EC):
                    nc.tensor.matmul(
                        ps[:, 0:D],
                        lhsT=vT_q[b // 4][:].rearrange("p (c f) -> p c f", c=EC)[
                            :, c, (b % 4) * 128 : (b % 4 + 1) * 128
                        ],
                        rhs=wvh_sb[:, c * D : (c + 1) * D],
                        start=(c == 0),
                        stop=(c == EC - 1),
                    )
                nc.vector.tensor_copy(
                    vh_sb[:, b * (D + 1) : b * (D + 1) + D], ps[:, 0:D]
                )

        wm = psS.tile([128, 512], F32, tag="psS", name="wm")
        for w in range(26):
            nc.tensor.matmul(
                wm[:, 0:128],
                lhsT=wq2_sb[:, 0:128],
                rhs=wq2_sb[:, 0:128],
                start=True,
                stop=True,
            )

        qproj_done = {0}
        qproj(0)
        khT_done = set()
        vh_done = set()

        flat = [(g, j) for g in range(NQT) for j in range(npairs[g])]
        pts = {}
        po_t = {}
        sc_cur = 0

        def scores(p):
            g, j = flat[p]
            if g not in qproj_done:
                qproj_done.add(g)
                qproj(g)
            t = j // 2
            if t not in khT_done:
                khT_done.add(t)
                khT_proj(t)
            pp = psP.tile([128, 1024], F32, tag="psP")
            nc.tensor.matmul(
                pp[:, 0:512],
                lhsT=khT2[0:64, j * 128 : (j + 1) * 128],
                rhs=qhT2[0:64, g * 512 : (g + 1) * 512],
                start=True,
                stop=True,
            )
            nc.tensor.matmul(
                pp[:, 512:1024],
                lhsT=khT2[64:128, j * 128 : (j + 1) * 128],
                rhs=qhT2[64:128, g * 512 : (g + 1) * 512],
                start=True,
                stop=True,
            )
            pt = ptpool.tile([128, 1024], BF, tag="pt")
            nc.scalar.activation(pt[:], pp[:], AF.Exp)
            if masked and j == npairs[g] - 1:
                nc.vector.tensor_mul(pt[:], pt[:], msk_sb[:])
            pts[p] = pt

        def av(p):
            g, j = flat[p]
            if j not in vh_done:
                vh_done.add(j)
                vh_pair(j)
            pt = pts.pop(p)
            po = po_t[g]
            nc.tensor.matmul(
                po[0:65, :],
                lhsT=vh_sb[:, (2 * j) * 65 : (2 * j + 1) * 65],
                rhs=pt[:, 0:512],
                start=(j == 0),
                stop=False,
            )
            nc.tensor.matmul(
                po[0:65, :],
                lhsT=vh_sb[:, (2 * j + 1) * 65 : (2 * j + 2) * 65],
                rhs=pt[:, 512:1024],
                start=False,
                stop=(j == npairs[g] - 1),
            )

        khT_done.add(0)
        khT_proj(0)
        vh_done.add(0)
        vh_pair(0)

        def tile_end(g):
            nc.vector.tensor_copy(
                ob[0:65, g * 512 : (g + 1) * 512], po_t[g][0:65, :]
            )
            nc.sync.dma_start(
                o[:, g * 512 : (g + 1) * 512],
                ob[0:65, g * 512 : (g + 1) * 512],
            )

        scores(0)
        sc_cur = 1

        for p, (g, j) in enumerate(flat):
            if j == 0:
                po_t[g] = psO.tile([128, 512], F32, tag="psO", name=f"po{g}")
            while sc_cur < min(p + 3, len(flat)):
                scores(sc_cur)
                sc_cur += 1
            av(p)
            if j == npairs[g] - 1:
                tile_end(g)

    _split_sync_waits(nc)
    return nc


_CACHE = {}


def _get_nc(causal):
    key = bool(causal)
    if key not in _CACHE:
        if causal:
            npairs = [g + 1 for g in range(NQT)]
        else:
            npairs = [NKB // 2] * NQT
        _CACHE[key] = build_nc(npairs, key)
    return _CACHE[key]


def kernel(q, k, v, mask, wq, wk, wv):
    q = np.asarray(q, np.float32)
    k = np.asarray(k, np.float32)
    v = np.asarray(v, np.float32)
    mask = np.asarray(mask)
    wq = np.asarray(wq, np.float32)
    wk = np.asarray(wk, np.float32)
    wv = np.asarray(wv, np.float32)

    m0 = mask[0]
    causal = bool(m0[0, 1] == 0)
    tril = np.tril(np.ones((S, S), np.int32))
    if causal:
        ok = np.array_equal(m0.astype(np.int32), tril)
    else:
        ok = bool((m0 != 0).all())
    if not ok:
        qh = q @ wq
        kh = k @ wk
        vh = v @ wv
        s = np.einsum("bqd,bkd->bqk", qh, kh) / np.sqrt(D)
        s = np.where(mask == 0, -np.inf, s)
        s = s - s.max(-1, keepdims=True)
        p = np.exp(s)
        p /= p.sum(-1, keepdims=True)
        return np.einsum("bqk,bkd->bqd", p, vh).astype(np.float32)

    nc = _get_nc(causal)

    def dup_w(w, scale=1.0):
        wc = (w * scale).reshape(EC, 128, D).astype(BF16NP)
        out = np.empty((128, EC, 128), dtype=BF16NP)
        out[:, :, 0:D] = wc.transpose(1, 0, 2)
        out[:, :, D:128] = wc.transpose(1, 0, 2)
        return np.ascontiguousarray(out.reshape(128, EC * 128))

    wq2 = dup_w(wq, 1.0 / np.sqrt(D))
    wk2 = dup_w(wk)
    wvh = np.ascontiguousarray(
        wv.reshape(EC, 128, D).transpose(1, 0, 2).reshape(128, EC * D)
    ).astype(BF16NP)

    def pack_tiles(x):
        nt = x.shape[0] // 512
        t4 = x.reshape(nt, 512, EC, 128)
        return np.ascontiguousarray(t4.transpose(0, 3, 2, 1)).reshape(
            nt, 128, EC * 512
        )

    in_maps = []
    for b in range(B):
        qT = pack_tiles(q[b].astype(BF16NP))
        for p in range(2):
            kb = k[b].reshape(32, 128, E)[p::2].reshape(SL, E)
            vb = v[b].reshape(32, 128, E)[p::2].reshape(SL, E)
            kT = pack_tiles(kb.astype(BF16NP))
            vT = pack_tiles(vb.astype(BF16NP))
            if causal:
                kk = np.arange(128)[:, None]
                qq = np.arange(512)[None, :]
                m0b = (qq >= kk + 128 * p).astype(BF16NP)
                m1b = (qq >= kk + 128 * p + 256).astype(BF16NP)
                mskd = np.concatenate([m0b, m1b], axis=1)
            else:
                mskd = np.ones((128, 1024), BF16NP)
            in_maps.append(
                {
                    "qt": qT,
                    "kt": kT,
                    "vt": vT,
                    "wq2": wq2,
                    "wk2": wk2,
                    "wvh": wvh,
                    "msk": mskd,
                }
            )

    globals()["_last_in_maps"] = in_maps
    res = run_bass_kernel_spmd(nc, in_maps, core_ids=list(range(8)))

    out = np.empty((B, S, D), np.float32)
    for b in range(B):
        oe = res.results[2 * b]["o"]
        oo = res.results[2 * b + 1]["o"]
        num = oe[0:D] + oo[0:D]
        den = oe[D : D + 1] + oo[D : D + 1]
        out[b] = (num / den).T
    return out
